# revision 21
# baseline (speedup 1.0000x reference)
"""Trainium2 Bass kernel for nn_Attention_78786880078278.

Dense causal multi-head attention layer (QKV proj + RoPE + causal softmax
attention + output proj), sharded over 8 NeuronCores:
  - NEFF 1 (head-parallel): each core computes QKV projections, RoPE and
    causal attention for its 2 heads (x 2 batches), producing per-head
    attention outputs.  QKV of batch/span (b,s) is software-pipelined with
    the attention chunks of earlier spans so the Tensor engine never idles
    (idle PE drops the clock from 2.4 GHz to the 1.2 GHz p-state).
  - host: pure relayout (gather + transpose) of the per-head outputs.
  - NEFF 2 (token-parallel): each core computes the output projection for
    its [token-half x hout-quarter] block with Wo stationary and tokens
    streaming (N=1024 matmuls), Wo/activation DMAs interleaved kt-uniformly.

All matmuls run in bf16 with fp32 PSUM accumulation (casts done on device;
constant tables are shipped bf16).  Host-side work is restricted to
slicing/transposition (no FLOPs).
"""

import contextlib
import ctypes
import hashlib
import json
import math
import os
import shutil
import sys
import types

import numpy as np
import ml_dtypes

# ---------------------------------------------------------------------------
# environment fixups
# ---------------------------------------------------------------------------

for _p in ("/opt/trn_rl_repo",):
    if _p not in sys.path and os.path.isdir(_p):
        sys.path.append(_p)

import concourse.bass as bass  # noqa: E402
import concourse.bass2jax as bass2jax  # noqa: E402
import concourse.mybir as mybir  # noqa: E402
import concourse.tile as tile  # noqa: E402
from concourse.bass_utils import run_bass_kernel_spmd  # noqa: E402

F32 = mybir.dt.float32
BF16 = mybir.dt.bfloat16
BF16_NP = ml_dtypes.bfloat16

_NEFF_CACHE_DIR = os.environ.get("NEFF_CACHE_DIR", "/tmp/neff_cache")


def _install_compile_fixups():
    """(1) Split multi-wait instructions: this walrus build encodes a single
    sync-wait slot per instruction and rejects Tile's final multi-wait drain.
    (2) Cache compiled NEFFs by BIR hash so repeated runs skip walrus."""
    if getattr(bass2jax, "_attn_fixup_installed", False):
        return
    orig = bass2jax.compile_bir_kernel

    def _fix_multiwait(bir_bytes):
        bir = json.loads(bir_bytes)
        changed = False
        for fn in bir.get("functions", []):
            for blk in fn.get("basic_blocks", fn.get("blocks", [])):
                new_insts = []
                for inst in blk.get("instructions", []):
                    si = inst.get("sync_info") or {}
                    waits = si.get("on_wait") or []
                    if len(waits) > 1:
                        changed = True
                        for i, w in enumerate(waits[:-1]):
                            pre = {
                                "name": f"{inst['name']}_w{i}",
                                "opcode": "Drain",
                                "engine": inst["engine"],
                                "ins": [],
                                "outs": [],
                                "sync_info": {"on_wait": [w], "on_update": []},
                            }
                            if "debug" in inst:
                                pre["debug"] = inst["debug"]
                            if "is_reset_sema" in inst:
                                pre["is_reset_sema"] = False
                            new_insts.append(pre)
                        si["on_wait"] = [waits[-1]]
                        inst["sync_info"] = si
                    new_insts.append(inst)
                blk["instructions"] = new_insts
        return json.dumps(bir).encode() if changed else bir_bytes

    def _patched(bir_json, tmpdir, neff_name="file.neff"):
        fixed = _fix_multiwait(bir_json)
        key = hashlib.sha256(fixed).hexdigest()[:24]
        cached = os.path.join(_NEFF_CACHE_DIR, f"{key}.neff")
        target = os.path.join(tmpdir, neff_name)
        if os.path.exists(cached):
            shutil.copy(cached, target)
            return target
        path = orig(fixed, tmpdir, neff_name)
        try:
            os.makedirs(_NEFF_CACHE_DIR, exist_ok=True)
            shutil.copy(path, cached)
        except OSError:
            pass
        return path

    bass2jax.compile_bir_kernel = _patched
    bass2jax._attn_fixup_installed = True


def _install_ntff_hook():
    """Register the NTFF profiling hook (used only when BASS_TRACE=1)."""
    try:
        import antenv
    except ImportError:
        return
    if "antenv.axon_hooks" in sys.modules:
        return
    so_path = "/opt/axon/libaxon_pjrt.so"
    try:
        lib = ctypes.CDLL(so_path)
    except OSError:
        return
    if not hasattr(lib, "axon_start_nrt_profile"):
        return
    lib.axon_start_nrt_profile.argtypes = [
        ctypes.POINTER(ctypes.c_int64),
        ctypes.c_size_t,
    ]
    lib.axon_start_nrt_profile.restype = ctypes.c_int64
    lib.axon_stop_nrt_profile.argtypes = [ctypes.c_char_p]
    lib.axon_stop_nrt_profile.restype = ctypes.c_int64

    @contextlib.contextmanager
    def _hook(output_dir, device_ids):
        import jax

        jax.devices()
        if device_ids:
            ids = (ctypes.c_int64 * len(device_ids))(*device_ids)
            rc = lib.axon_start_nrt_profile(ids, len(device_ids))
        else:
            rc = lib.axon_start_nrt_profile(None, 0)
        if rc != 0:
            raise RuntimeError(f"axon_start_nrt_profile rc={rc}")
        try:
            yield
        finally:
            n = lib.axon_stop_nrt_profile(str(output_dir).encode())
            print(f"profile: {n} file(s) in {output_dir}", file=sys.stderr)

    mod = types.ModuleType("antenv.axon_hooks")
    mod.get_axon_ntff_profile_hook = lambda: _hook
    mod.set_axon_ntff_profile_hook = lambda h: None
    sys.modules["antenv.axon_hooks"] = mod
    antenv.axon_hooks = mod


_install_compile_fixups()
_install_ntff_hook()

# ---------------------------------------------------------------------------
# problem constants (hardcoded per the task spec)
# ---------------------------------------------------------------------------

HIDDEN = 2048
HEADS = 16
HD = 128  # head dim
B = 2
S = 2048
N_CORES = 8
HPC = HEADS // N_CORES  # heads per core = 2
SPAN = 512
NSPANS = S // SPAN  # 4 query spans per batch
KT = HIDDEN // 128  # 16 contraction tiles
TT = S // 128  # 16 token tiles per batch
SCALE = 1.0 / math.sqrt(HD)
SCALE_LN2 = SCALE  # exp() path; kept for clarity

LAST_RESULTS = []  # BassKernelResults of the most recent kernel() call


# ---------------------------------------------------------------------------
# NEFF 1: QKV projections + RoPE + causal attention for 2 heads x 2 batches
# ---------------------------------------------------------------------------

def build_attn_nc():
    nc = bass.Bass(target_bir_lowering=False, debug=False)

    xT = nc.dram_tensor("xT", [B, HIDDEN, S], BF16, kind="ExternalInput")
    wqT = nc.dram_tensor("wqT", [HIDDEN, HPC * HD], BF16, kind="ExternalInput")
    wkT = nc.dram_tensor("wkT", [HIDDEN, HPC * HD], BF16, kind="ExternalInput")
    wvT = nc.dram_tensor("wvT", [HIDDEN, HPC * HD], BF16, kind="ExternalInput")
    cosT = nc.dram_tensor("cosT", [HD, S], BF16, kind="ExternalInput")
    sinT = nc.dram_tensor("sinT", [HD, S], BF16, kind="ExternalInput")  # signed
    maskd = nc.dram_tensor("mask", [128, 128], BF16, kind="ExternalInput")
    attnout = nc.dram_tensor(
        "attnout", [B, HPC, TT, 128, 128], BF16, kind="ExternalOutput"
    )

    with tile.TileContext(nc) as tc:
        with (
            tc.tile_pool(name="persist", bufs=1) as persist,
            tc.tile_pool(name="xspan", bufs=2) as xspan_p,
            tc.tile_pool(name="rope", bufs=3) as rope,
            tc.tile_pool(name="epool", bufs=17) as epool,
            tc.tile_pool(name="opool", bufs=2) as opool,
            tc.tile_pool(name="rpool", bufs=4) as rpool,
            tc.tile_pool(name="ps_qk", bufs=2, space="PSUM") as ps_qk,
            tc.tile_pool(name="ps_sc", bufs=2, space="PSUM") as ps_sc,
            tc.tile_pool(name="ps_o", bufs=2, space="PSUM") as ps_o,
        ):
            # ---------------- persistent tiles ----------------
            wq_bf = persist.tile([128, KT, HPC * HD], BF16, tag="wq_bf")
            wk_bf = persist.tile([128, KT, HPC * HD], BF16, tag="wk_bf")
            wv_bf = persist.tile([128, KT, HPC * HD], BF16, tag="wv_bf")
            cos_sb = persist.tile([HD, S], BF16, tag="cos_sb")
            sin_sb = persist.tile([HD, S], BF16, tag="sin_sb")
            mask_bf = persist.tile([128, 128], BF16, tag="mask_bf")
            q_sb = persist.tile([HD, B, HPC, S], BF16, tag="q_sb")
            k_sb = persist.tile([HD, B, HPC, S], BF16, tag="k_sb")
            # v with an appended ones column (denominator trick)
            v_sb = persist.tile([128, B, TT, HPC, HD + 1], BF16, tag="v_sb")

            def load_w_quarter(wdram, wbf, p):
                src = wdram[p * 512:(p + 1) * 512, :]
                nc.sync.dma_start(
                    wbf[:, p * (KT // 4):(p + 1) * (KT // 4), :],
                    src.rearrange("(ko p) h -> p ko h", p=128),
                )

            def load_xspan(b, span, quarters=1, interleave=None):
                """DMA one 512-token span of x straight into SBUF (bf16 over
                the wire).  `quarters`>1 splits the DMA for finer-grained
                readiness during the DMA-bound prologue."""
                xspan = xspan_p.tile([128, KT, SPAN], BF16, tag="x_bf")
                kq = KT // quarters
                for quarter in range(quarters):
                    src = xT[
                        b,
                        quarter * kq * 128:(quarter + 1) * kq * 128,
                        span * SPAN:(span + 1) * SPAN,
                    ]
                    nc.sync.dma_start(
                        xspan[:, quarter * kq:(quarter + 1) * kq, :],
                        src.rearrange("(ko p) t -> p ko t", p=128),
                    )
                    if interleave is not None:
                        interleave(quarter)
                return xspan

            def qkv_span(b, span, xspan):
                sl = slice(span * SPAN, (span + 1) * SPAN)
                # V first: its PSUM->v_sb drains (DVE) enqueue ahead of the
                # RoPE ops so the next attention chunk's attn@V never waits
                # on a head-of-line-blocked DVE queue.
                for j in range(4):
                    tt = span * 4 + j
                    psv = ps_qk.tile([128, SPAN], F32, tag="qk")
                    for kt in range(KT):
                        nc.tensor.matmul(
                            psv[:, 0:HPC * HD],
                            xspan[:, kt, j * 128:(j + 1) * 128],
                            wv_bf[:, kt, :],
                            start=(kt == 0),
                            stop=(kt == KT - 1),
                        )
                    for h in range(HPC):
                        nc.vector.tensor_copy(
                            v_sb[:, b, tt, h, 0:HD],
                            psv[:, h * HD:(h + 1) * HD],
                        )

                for h in range(HPC):
                    hsl = slice(h * HD, (h + 1) * HD)
                    for wbf, dst in ((wq_bf, q_sb), (wk_bf, k_sb)):
                        ps = ps_qk.tile([128, SPAN], F32, tag="qk")
                        for kt in range(KT):
                            nc.tensor.matmul(
                                ps[:],
                                wbf[:, kt, hsl],
                                xspan[:, kt, :],
                                start=(kt == 0),
                                stop=(kt == KT - 1),
                            )
                        # RoPE: out = p*cos + rot(p)*sin_signed, all bf16 on
                        # DVE (2x 16-bit throughput). ScalarE drains PSUM
                        # three ways — straight and partition-swapped (legal
                        # because the source is PSUM) — so no DMA sits in
                        # the rotate path at all.
                        pf = rope.tile([128, SPAN], BF16, tag="pf")
                        rot = rope.tile([128, SPAN], BF16, tag="rot")
                        nc.scalar.copy(pf[:], ps[:])
                        nc.scalar.copy(rot[0:64, :], ps[64:128, :])
                        nc.scalar.copy(rot[64:128, :], ps[0:64, :])
                        nc.vector.tensor_mul(pf[:], pf[:], cos_sb[:, sl])
                        nc.vector.tensor_mul(rot[:], rot[:], sin_sb[:, sl])
                        nc.vector.tensor_add(dst[:, b, h, sl], pf[:], rot[:])

            def attn_scores(b, h, s):
                qsl = slice(s * SPAN, (s + 1) * SPAN)
                nkt = 4 * s + 4  # causal: k tiles 0 .. 4s+3 (always even)
                es = []
                for kp in range(nkt // 2):
                    # two k-tiles share a 2-bank PSUM: one ACTIVATE covers both
                    # exps, amortizing ScalarE's per-op overhead
                    psc = ps_sc.tile([128, 2 * SPAN], F32, tag="sc")
                    for half in range(2):
                        kt = 2 * kp + half
                        nc.tensor.matmul(
                            psc[:, half * SPAN:(half + 1) * SPAN],
                            k_sb[:, b, h, kt * 128:(kt + 1) * 128],
                            q_sb[:, b, h, qsl],
                            start=True,
                            stop=True,
                        )
                    e2 = epool.tile([128, 2 * SPAN], BF16, tag="e")
                    nc.scalar.activation(
                        e2[:], psc[:], mybir.ActivationFunctionType.Exp, scale=SCALE
                    )
                    for half in range(2):
                        kt = 2 * kp + half
                        jd = kt - 4 * s
                        base = half * SPAN
                        if jd >= 0:  # diagonal block: zero out k > q
                            nc.vector.tensor_mul(
                                e2[:, base + jd * 128:base + (jd + 1) * 128],
                                e2[:, base + jd * 128:base + (jd + 1) * 128],
                                mask_bf[:],
                            )
                        es.append(e2[:, base:base + SPAN])
                return es

            def attn_vmm(b, h, s, es, split_store=False):
                o_sb = opool.tile([128, 4, 128], BF16, tag="o")
                for j in range(4):
                    last_kt = 4 * s + j
                    pso = ps_o.tile([128, HD + 1], F32, tag="o")
                    for kt in range(last_kt + 1):
                        nc.tensor.matmul(
                            pso[:],
                            es[kt][:, j * 128:(j + 1) * 128],
                            v_sb[:, b, kt, h, :],
                            start=(kt == 0),
                            stop=(kt == last_kt),
                        )
                    recip = rpool.tile([128, 1], F32, tag="recip")
                    nc.vector.reciprocal(recip[:], pso[:, HD:HD + 1])
                    nc.vector.tensor_scalar_mul(
                        o_sb[:, j, :], pso[:, 0:HD], recip[:]
                    )
                    if split_store:
                        # tail chunks: store per query tile so the last DMA
                        # transfer starts as early as possible
                        dst = attnout[b, h, 4 * s + j, :, :]
                        nc.sync.dma_start(dst, o_sb[:, j, :])
                if not split_store:
                    dst = attnout[b, h, 4 * s:4 * s + 4, :, :]
                    # Sync-queue store: HWDGE issue is ~0.6us vs ~0.9 on the
                    # GpSimd SWDGE path, and the queue is otherwise idle here
                    nc.sync.dma_start(
                        dst.rearrange("qt ql dl -> ql qt dl"), o_sb[:]
                    )

            # ---------------- emission schedule ----------------
            # Startup is DMA-bound: interleave the first x span's quarters
            # with the wq quarters so the first q matmuls start ~3us in, then
            # stream wk/wv/tables behind them while the PE is busy.
            x00 = load_xspan(
                0, 0, quarters=4,
                interleave=lambda p: load_w_quarter(wvT, wv_bf, p),
            )
            for p in range(4):
                load_w_quarter(wqT, wq_bf, p)
            for p in range(4):
                load_w_quarter(wkT, wk_bf, p)
            nc.sync.dma_start(cos_sb[:], cosT[:])
            nc.sync.dma_start(sin_sb[:], sinT[:])
            nc.sync.dma_start(mask_bf[:], maskd[:])
            nc.vector.memset(v_sb[:, :, :, :, HD], 1.0)

            # Software pipeline over spans: attention of span s starts right
            # after qkv of (b, s); one qkv span is emitted per attention
            # chunk so the PE always has projection matmuls to chew on while
            # ScalarE computes the exps of the next chunk.  Batch 1's chunks
            # run span-descending so the final (tail) chunk is the smallest.
            qkv_span(0, 0, x00)
            chunks = [(0, s, h) for s in range(NSPANS) for h in range(HPC)]
            chunks += [(1, s, h) for s in reversed(range(NSPANS))
                       for h in range(HPC)]
            qkv_queue = [(b, s) for b in range(B) for s in range(NSPANS)][1:]
            es_map = {0: attn_scores(chunks[0][0], chunks[0][2], chunks[0][1])}
            for i in range(len(chunks)):
                if qkv_queue:
                    qb, qs = qkv_queue.pop(0)
                    qkv_span(qb, qs, load_xspan(qb, qs))
                if i + 1 < len(chunks):
                    cb, cs, ch = chunks[i + 1]
                    es_map[i + 1] = attn_scores(cb, ch, cs)
                cb, cs, ch = chunks[i]
                attn_vmm(cb, ch, cs, es_map.pop(i),
                         split_store=(i >= len(chunks) - 2))
    return nc


# ---------------------------------------------------------------------------
# NEFF 2: output projection, token-parallel
# ---------------------------------------------------------------------------

def build_oproj_nc():
    """outT[hout, tok] = WoT.T-tiles stationary, token columns streaming, on a
    2x4 (token-half x hout-quarter) core grid.  kt is the outer loop so the
    Wo and activation DMAs interleave uniformly (the whole NEFF is near the
    DMA roofline); N=1024 matmuls keep the PE p-state at 2.4 GHz.
    """
    nc = bass.Bass(target_bir_lowering=False, debug=False)

    TOKS = (B * S) // 2   # 2048 tokens per core (token half)
    HOUT = HIDDEN // 4    # 512 output channels per core (hout quarter)
    HALF = TOKS // 2      # 1024-token accumulators
    attnT = nc.dram_tensor("attnT", [HIDDEN, TOKS], BF16, kind="ExternalInput")
    woT = nc.dram_tensor("woT", [HIDDEN, HOUT], BF16, kind="ExternalInput")
    out = nc.dram_tensor("out", [HOUT, TOKS], BF16, kind="ExternalOutput")

    with tile.TileContext(nc) as tc:
        with (
            tc.tile_pool(name="persist", bufs=1) as persist,
            tc.tile_pool(name="outp", bufs=4) as outp,
            tc.tile_pool(name="psum", bufs=4, space="PSUM") as psum,
        ):
            wo_bf = persist.tile([128, KT, HOUT], BF16, tag="wo_bf")
            # the whole activation block stays resident (64 KiB/partition);
            # token-half slices stream in per kt so the DMA need is uniform
            a_sb = persist.tile([128, KT, TOKS], BF16, tag="a_sb")

            def load_wo(q, kts=None):
                k0, k1 = (q * 4, (q + 1) * 4) if kts is None else kts
                src = woT[k0 * 128:k1 * 128, :]
                nc.sync.dma_start(
                    wo_bf[:, k0:k1, :],
                    src.rearrange("(ko p) h -> p ko h", p=128),
                )

            def load_a(kt, half):
                nc.sync.dma_start(
                    a_sb[:, kt, half * HALF:(half + 1) * HALF],
                    attnT[kt * 128:(kt + 1) * 128,
                          half * HALF:(half + 1) * HALF],
                )

            # prefetch so the PE, once started, never catches the
            # (roofline-limited) DMA stream; first wo slice is kt=0 only so
            # the first matmul issues ~1.5us earlier
            load_wo(0, kts=(0, 1))
            load_a(0, 0)
            load_wo(0, kts=(1, 4))
            load_a(1, 0)
            load_wo(1)
            load_a(2, 0)

            for half in range(2):
                accs = [psum.tile([128, HALF], F32, tag="ps", name=f"ps_{half}_{o}")
                        for o in range(4)]
                for kt in range(KT):
                    if half == 0:
                        if kt + 3 < KT:
                            load_a(kt + 3, 0)
                        if kt == 4:
                            load_wo(2)
                        elif kt == 8:
                            load_wo(3)
                        elif kt >= 10 and kt < 14:
                            load_a(kt - 10, 1)  # head start on half 2
                    else:
                        if kt + 4 < KT:
                            load_a(kt + 4, 1)
                    for o in range(4):
                        # a matmul may write at most one PSUM bank (512 f32
                        # cols): two 512-wide streams per accumulator
                        for sub in range(2):
                            ssl = slice(sub * 512, (sub + 1) * 512)
                            nc.tensor.matmul(
                                accs[o][:, ssl],
                                wo_bf[:, kt, o * 128:(o + 1) * 128],
                                a_sb[:, kt, half * HALF + sub * 512:
                                     half * HALF + (sub + 1) * 512],
                                start=(kt == 0),
                                stop=(kt == KT - 1),
                            )
                for o in range(4):
                    # alternate ScalarE/DVE so the four drains run pairwise
                    # in parallel; output DMAs ride the (idle) Sync queue
                    ob = outp.tile([128, HALF], BF16, tag="o")
                    if o % 2 == 0:
                        nc.scalar.copy(ob[:], accs[o][:])
                    else:
                        nc.vector.tensor_copy(ob[:], accs[o][:])
                    nc.sync.dma_start(
                        out[o * 128:(o + 1) * 128, half * HALF:(half + 1) * HALF],
                        ob[:],
                    )
    return nc


# ---------------------------------------------------------------------------
# host driver
# ---------------------------------------------------------------------------

_NC_CACHE = {}


def _get_ncs():
    if "attn" not in _NC_CACHE:
        _NC_CACHE["attn"] = build_attn_nc()
        _NC_CACHE["oproj"] = build_oproj_nc()
    return _NC_CACHE["attn"], _NC_CACHE["oproj"]


def _rope_tables():
    inv_freq = 1.0 / (10000.0 ** (np.arange(0, HD, 2, dtype=np.float32) / HD))
    t = np.arange(S, dtype=np.float32)
    freqs = np.einsum("i,j->ij", t, inv_freq)  # [S, HD/2]
    emb = np.concatenate([freqs, freqs], axis=-1)  # [S, HD]
    cos = np.cos(emb).astype(np.float32)
    sin = np.sin(emb).astype(np.float32)
    cosT = np.ascontiguousarray(cos.T)  # [HD, S]
    sinT = np.ascontiguousarray(sin.T)
    sinT_signed = sinT.copy()
    sinT_signed[0:64, :] *= -1.0  # fold rotate_half's negation into the table
    return cosT.astype(BF16_NP), sinT_signed.astype(BF16_NP)


def kernel(x, Wq, Wk, Wv, Wo):
    x = np.asarray(x, dtype=np.float32)
    Wq = np.asarray(Wq, dtype=np.float32)
    Wk = np.asarray(Wk, dtype=np.float32)
    Wv = np.asarray(Wv, dtype=np.float32)
    Wo = np.asarray(Wo, dtype=np.float32)

    nc1, nc2 = _get_ncs()
    core_ids = list(range(N_CORES))
    trace = bool(os.environ.get("BASS_TRACE"))

    cosT, sinT_signed = _rope_tables()
    # mask[k,q]=1 iff k<=q
    mask = np.triu(np.ones((128, 128), dtype=np.float32)).astype(BF16_NP)
    # bf16 over the wire: identical rounding to the on-device cast it
    # replaces, at half the (roofline-limited) DMA bytes
    xT = np.ascontiguousarray(x.transpose(0, 2, 1)).astype(BF16_NP)

    in_maps1 = []
    for c in range(N_CORES):
        csl = slice(c * HPC * HD, (c + 1) * HPC * HD)
        in_maps1.append(
            {
                "xT": xT,
                "wqT": np.ascontiguousarray(Wq[csl, :].T).astype(BF16_NP),
                "wkT": np.ascontiguousarray(Wk[csl, :].T).astype(BF16_NP),
                "wvT": np.ascontiguousarray(Wv[csl, :].T).astype(BF16_NP),
                "cosT": cosT,
                "sinT": sinT_signed,
                "mask": mask,
            }
        )

    LAST_RESULTS.clear()
    res1 = run_bass_kernel_spmd(nc1, in_maps1, core_ids=core_ids, trace=trace)
    LAST_RESULTS.append(res1)

    # host relayout: per-head attention outputs -> attnT [HIDDEN, B*S]
    arr = np.stack([res1.results[c]["attnout"] for c in range(N_CORES)])
    # axes: (core, b, h, qt, ql, dl) -> d = core*256 + h*128 + dl,
    #       tok = b*2048 + qt*128 + ql
    attnT = np.ascontiguousarray(
        arr.transpose(0, 2, 5, 1, 3, 4).reshape(HIDDEN, B * S)
    )
    woT = np.ascontiguousarray(Wo.T).astype(BF16_NP)

    TOKS = (B * S) // 2
    HOUT = HIDDEN // 4
    in_maps2 = []
    for c in range(N_CORES):
        ti, hj = c // 4, c % 4
        in_maps2.append(
            {
                "attnT": np.ascontiguousarray(attnT[:, ti * TOKS:(ti + 1) * TOKS]),
                "woT": np.ascontiguousarray(woT[:, hj * HOUT:(hj + 1) * HOUT]),
            }
        )
    res2 = run_bass_kernel_spmd(nc2, in_maps2, core_ids=core_ids, trace=trace)
    LAST_RESULTS.append(res2)

    out = np.empty((B * S, HIDDEN), dtype=np.float32)
    for c in range(N_CORES):
        ti, hj = c // 4, c % 4
        out[ti * TOKS:(ti + 1) * TOKS, hj * HOUT:(hj + 1) * HOUT] = (
            res2.results[c]["out"].T.astype(np.float32)
        )
    return np.ascontiguousarray(out.reshape(B, S, HIDDEN), dtype=np.float32)
